# revision 2
# baseline (speedup 1.0000x reference)
"""Bass/Tile TRN2 kernel v2 for nn_AttentionLayer (B=2, S=2048, D=1024, H=16).

Sharding: 8 cores = 2 (batch) x 4 (head groups of 4 heads each).

v2 design (vs v1 baseline):
  - All matmul operands bf16 (x, W, Q^T, K^T, V', e); fp32 PSUM/bias/norm.
  - QK: row-tiled pairs (K=64 per head at tile rows 0/64) -> 2 heads
    stream concurrently through the PE.
  - PV: e (exp of scores, bf16) is the STATIONARY operand [sk=128, sq=128
    slice]; V' [sk, 65] (64 V cols + ones col) is the mover (N=65).
    Output lands pre-transposed [sq, d+1] in PSUM with the softmax
    denominator in col 64; accumulated over all 16 sk chunks in PSUM.
  - Drain: reciprocal of col 64 * cols 0:63 on DVE, DMA out. No PE
    transposes.
  - PSUM: scores 2x[128,1024] (4 banks) + pv 3x[128,4,128] (3 banks) +
    proj 1x[128,512] (1 bank) = 8 banks.
  - Schedule: blocks A=(0,0), B=(1,0) chase K/V chunk loads; B's h1 PV
    deferred (e tiles buffered) until A drains to stay in 3 pv slots.
"""

import os
import sys

sys.path.insert(0, "/opt/trn_rl_repo")

import numpy as np

import concourse.bacc as bacc
import concourse.mybir as mybir
from concourse.tile import TileContext
from concourse.bass_utils import run_bass_kernel_spmd

F32 = mybir.dt.float32
BF16 = mybir.dt.bfloat16
AF = mybir.ActivationFunctionType
ALU = mybir.AluOpType

B, S, D, H = 2, 2048, 1024, 16
HD = D // H            # 64
NCORES = 8
HPC = 4                # heads per core
E = HPC * HD           # 256 output cols per core
EV = HPC * (HD + 1)    # 260: V' with ones column per head
DCH = D // 128         # 8 d chunks
SKC = S // 128         # 16 sk chunks
SCALE = 1.0 / np.sqrt(HD)


def build_kernel(loop_n: int = 1):
    nc = bacc.Bacc()
    # x pre-chunked host-side to [p, si, c, s'] so each si-load is one
    # 8KB-contiguous run per partition; weights pre-arranged likewise.
    xqT = nc.dram_tensor("xqT", [128, 4, DCH, 512], BF16, kind="ExternalInput")
    xkT = nc.dram_tensor("xkT", [128, 4, DCH, 512], BF16, kind="ExternalInput")
    xvT = nc.dram_tensor("xvT", [128, 4, DCH, 512], BF16, kind="ExternalInput")
    wq = nc.dram_tensor("wq", [128, DCH, E], BF16, kind="ExternalInput")
    wk = nc.dram_tensor("wk", [128, DCH, E], BF16, kind="ExternalInput")
    wv = nc.dram_tensor("wv", [128, DCH, EV], BF16, kind="ExternalInput")
    bq = nc.dram_tensor("bq", [128, 2], F32, kind="ExternalInput")
    bk = nc.dram_tensor("bk", [128, 2], F32, kind="ExternalInput")
    bv = nc.dram_tensor("bv", [128, EV], F32, kind="ExternalInput")
    # out as [hh, p, t, d] so each (block, head) store is one DMA with
    # 1KB-contiguous runs per partition; host re-gathers.
    out = nc.dram_tensor("out", [HPC, 128, 16, HD], F32, kind="ExternalOutput")

    with TileContext(nc) as tc:
        with tc.tile_pool(name="wsb", bufs=1) as wsb, \
             tc.tile_pool(name="xsb", bufs=3) as xsb, \
             tc.tile_pool(name="qkv", bufs=1) as qkv, \
             tc.tile_pool(name="esb", bufs=32) as esb, \
             tc.tile_pool(name="osb", bufs=8) as osb, \
             tc.tile_pool(name="pps", bufs=2, space="PSUM") as pps, \
             tc.tile_pool(name="stp", bufs=2, space="PSUM") as stp, \
             tc.tile_pool(name="pvp", bufs=2, space="PSUM") as pvp:

            # ---- weights / constants (loads ordered for the pipeline) ----
            wk_t = wsb.tile([128, DCH, E], BF16)
            nc.sync.dma_start(wk_t[:], wk[:])
            bk_t = wsb.tile([128, 2], F32)
            nc.sync.dma_start(bk_t[:], bk[:])
            # touch Exp early so the ACT table load happens during proj
            warm = wsb.tile([128, 1], F32)
            nc.scalar.activation(warm[:], bk_t[:, 0:1], AF.Exp)

            def load_wq():
                wq_t = wsb.tile([128, DCH, E], BF16, name="wq_t")
                nc.sync.dma_start(wq_t[:], wq[:])
                bq_t = wsb.tile([128, 2], F32, name="bq_t")
                nc.sync.dma_start(bq_t[:], bq[:])
                return wq_t, bq_t

            def load_wv():
                wv_t = wsb.tile([128, DCH, EV], BF16, name="wv_t")
                nc.sync.dma_start(wv_t[:], wv[:])
                bv_t = wsb.tile([128, EV], F32, name="bv_t")
                nc.sync.dma_start(bv_t[:], bv[:])
                return wv_t, bv_t

            def load_x(src, si):
                x_t = xsb.tile([128, DCH, 512], BF16, tag="x", name=f"x_{si}")
                nc.sync.dma_start(x_t[:], src[:, si])
                return x_t



            def project_qk_et(x_t, w_t, b_t, o_t, si, et, kt=False):
                """One et half (128 output cols) of a Q/K projection chunk."""
                sl = slice(512 * si, 512 * (si + 1))
                ps = pps.tile([128, 512], F32, tag="pj", name="ps_qk")
                for c in range(DCH):
                    nc.tensor.matmul(
                        ps[:], w_t[:, c, 128 * et:128 * (et + 1)],
                        x_t[:, c], start=(c == 0), stop=(c == DCH - 1))
                if kt:
                    nc.vector.tensor_scalar(
                        out=o_t[0:64, 2 * et, sl], in0=ps[0:64, :],
                        scalar1=b_t[0:64, et:et + 1], scalar2=None,
                        op0=ALU.add)
                    nc.vector.tensor_scalar(
                        out=o_t[64:128, 2 * et + 1, sl], in0=ps[64:128, :],
                        scalar1=b_t[64:128, et:et + 1], scalar2=None,
                        op0=ALU.add)
                else:
                    nc.vector.tensor_scalar(
                        out=o_t[:, et, sl], in0=ps[:],
                        scalar1=b_t[:, et:et + 1], scalar2=None,
                        op0=ALU.add)

            def project_v_k(x_t, wv_t, bv_t, si, k):
                """One k quarter (128 s rows = sk chunk 4*si+k) of V'."""
                psv = pps.tile([128, EV], F32, tag="pj", name="ps_v")
                for c in range(DCH):
                    nc.tensor.matmul(
                        psv[:], x_t[:, c, 128 * k:128 * (k + 1)],
                        wv_t[:, c], start=(c == 0), stop=(c == DCH - 1))
                nc.vector.tensor_tensor(
                    out=V_t[:, 4 * si + k, :], in0=psv[:], in1=bv_t[:],
                    op=ALU.add)

            import contextlib

            def body_scope():
                if loop_n > 1:
                    return tc.For_i(0, loop_n, 1)
                return contextlib.nullcontext()

            with body_scope():
                QT_t = qkv.tile([128, 2, S], BF16, tag="QT", name="QT_t")
                KT_t = qkv.tile([128, 4, S], BF16, tag="KT", name="KT_t")
                V_t = qkv.tile([128, SKC, EV], BF16, tag="V", name="V_t")

                from collections import deque

                BLKS = [(0, 0), (1, 0), (0, 1), (1, 1),
                        (0, 2), (1, 2), (0, 3), (1, 3)]
                order = [(pr, sqt, h) for (pr, sqt) in BLKS for h in (0, 1)]
                gq = {key: deque() for key in order}
                pvs = {}
                st = {"oi": 0, "pi": 0}
                npop = {key: 0 for key in order}

                def open_next():
                    key = order[st["oi"]]
                    pvs[key] = pvp.tile([128, 4, 128], F32, tag="pv",
                                        name=f"pv{key[0]}{key[1]}{key[2]}")
                    st["oi"] += 1

                def qk_exp(pr, sqt, cpl):
                    sq0 = 512 * sqt
                    sts = []
                    for h in (0, 1):
                        st_t = stp.tile([128, 1024], F32, tag="st",
                                        name=f"st{h}")
                        sts.append(st_t)
                    for q in (0, 1):
                        ck = 2 * cpl + q
                        for h in (0, 1):
                            pl = slice(64 * h, 64 * (h + 1))
                            nc.tensor.matmul(
                                sts[h][:, 512 * q:512 * (q + 1)],
                                KT_t[pl, 2 * pr + h, 128 * ck:128 * (ck + 1)],
                                QT_t[pl, pr, sq0:sq0 + 512],
                                start=True, stop=True)
                    es = []
                    for h in (0, 1):
                        e_t = esb.tile([128, 1024], BF16, tag="e", name="e_t")
                        nc.scalar.activation(e_t[:], sts[h][:], AF.Exp,
                                             scale=float(SCALE))
                        es.append(e_t)
                    return es

                def pv_group(pr, sqt, h, cpl, e_t):
                    """8 PV matmuls for one (block, head, chunk-pair).
                    has_written clear (start) is bank-granular: only the
                    bank's first MM sets it."""
                    hh = 2 * pr + h
                    pv = pvs[(pr, sqt, h)]
                    for q in (0, 1):
                        ck = 2 * cpl + q
                        for s in range(4):
                            nc.tensor.matmul(
                                pv[:, s, 0:65],
                                e_t[:, 512 * q + 128 * s:512 * q + 128 * s + 128],
                                V_t[:, ck, 65 * hh:65 * hh + 65],
                                start=(cpl == 0 and q == 0 and s == 0),
                                stop=(cpl == 7 and q == 1),
                                skip_group_check=True)

                def drain(key):
                    pv = pvs.pop(key)
                    pr, sqt, h = key
                    hh = 2 * pr + h
                    obs = osb.tile([128, 4, HD], F32, tag="ob", name="obs")
                    for s in range(4):
                        rc = osb.tile([128, 1], F32, tag="rc", name="rc")
                        nc.vector.reciprocal(rc[:], pv[:, s, 64:65])
                        nc.vector.tensor_scalar(
                            out=obs[:, s, :], in0=pv[:, s, 0:HD],
                            scalar1=rc[:], scalar2=None, op0=ALU.mult)
                    nc.sync.dma_start(
                        out[hh, :, 4 * sqt:4 * sqt + 4, :], obs[:])

                def pump(n, allowed_cpl=7):
                    """Emit up to n pending PV groups, strictly in ring
                    order; drain + open-next when a (block, head) finishes."""
                    popped = 0
                    while popped < n and st["pi"] < len(order):
                        key = order[st["pi"]]
                        q = gq[key]
                        if not q or q[0][0] > allowed_cpl:
                            break
                        cpl, e_t = q.popleft()
                        pv_group(key[0], key[1], key[2], cpl, e_t)
                        npop[key] += 1
                        popped += 1
                        if npop[key] == 8:
                            drain(key)
                            st["pi"] += 1
                            if st["oi"] < len(order):
                                open_next()

                def pump_all():
                    while st["pi"] < len(order) and gq[order[st["pi"]]]:
                        pump(1000)
                        if st["pi"] < len(order) and not gq[order[st["pi"]]]:
                            break

                def do_cpl(pr, sqt, cpl):
                    es = qk_exp(pr, sqt, cpl)
                    gq[(pr, sqt, 0)].append((cpl, es[0]))
                    gq[(pr, sqt, 1)].append((cpl, es[1]))

                def proj_q(x_t, si):
                    project_qk_et(x_t, wq_t, bq_t, QT_t, si, 0)
                    project_qk_et(x_t, wq_t, bq_t, QT_t, si, 1)

                # ---- head: K0/Q0 so the first QK fires early ----
                xk = load_x(xkT, 0)
                project_qk_et(xk, wk_t, bk_t, KT_t, 0, 0, kt=True)
                wq_t, bq_t = load_wq()
                xq0 = load_x(xqT, 0)
                project_qk_et(xq0, wq_t, bq_t, QT_t, 0, 0)
                open_next()  # (0,0,0)
                open_next()  # (0,0,1)
                do_cpl(0, 0, 0)
                project_qk_et(xk, wk_t, bk_t, KT_t, 0, 1, kt=True)
                project_qk_et(xq0, wq_t, bq_t, QT_t, 0, 1)
                wv_t, bv_t = load_wv()
                xv = load_x(xvT, 0)

                # ---- load phase: A/B/C QK+exp chase the K/V chunk loads;
                # PV pumps trail behind the V' projections ----
                do_cpl(1, 0, 0)
                xq1 = load_x(xqT, 1)
                project_v_k(xv, wv_t, bv_t, 0, 0)
                project_v_k(xv, wv_t, bv_t, 0, 1)
                pump(2, 0)
                do_cpl(0, 0, 1)
                pump(2, 0)
                proj_q(xq1, 1)
                do_cpl(1, 0, 1)
                project_v_k(xv, wv_t, bv_t, 0, 2)
                project_v_k(xv, wv_t, bv_t, 0, 3)
                pump(2, 1)
                do_cpl(0, 1, 0)
                pump(2, 1)
                do_cpl(0, 1, 1)
                pump(2, 1)

                for si in (1, 2, 3):
                    xk = load_x(xkT, si)
                    project_qk_et(xk, wk_t, bk_t, KT_t, si, 0, kt=True)
                    project_qk_et(xk, wk_t, bk_t, KT_t, si, 1, kt=True)
                    xv = load_x(xvT, si)
                    do_cpl(0, 0, 2 * si)
                    pump(2, 2 * si - 1)
                    project_v_k(xv, wv_t, bv_t, si, 0)
                    project_v_k(xv, wv_t, bv_t, si, 1)
                    do_cpl(1, 0, 2 * si)
                    pump(2, 2 * si)
                    do_cpl(0, 1, 2 * si)
                    pump(2, 2 * si)
                    do_cpl(0, 0, 2 * si + 1)
                    pump(2, 2 * si)
                    project_v_k(xv, wv_t, bv_t, si, 2)
                    project_v_k(xv, wv_t, bv_t, si, 3)
                    do_cpl(1, 0, 2 * si + 1)
                    pump(3, 2 * si + 1)
                    do_cpl(0, 1, 2 * si + 1)
                    pump(3, 2 * si + 1)

                # ---- tail segments: C finishes, then D..H; pumps drain the
                # previous block while the current one feeds ACT ----
                def seg(blk, cap=4, projq=None):
                    for c in range(8):
                        do_cpl(blk[0], blk[1], c)
                        pump(cap)
                        if projq is not None and c == 4:
                            xt = load_x(xqT, projq)
                            proj_q(xt, projq)
                seg((1, 1), projq=2)
                seg((0, 2), projq=3)
                seg((1, 2))
                seg((0, 3), cap=5)
                seg((1, 3), cap=5)
                pump_all()
    nc.compile()
    return nc


_NC_CACHE = {}


def _get_nc(repeat: int = 1, loop_n: int = 1):
    key = (repeat, loop_n)
    if key not in _NC_CACHE:
        _NC_CACHE[key] = build_kernel(max(repeat, loop_n))
    return _NC_CACHE[key]


def _to_bf16(a):
    import ml_dtypes
    return np.asarray(a, np.float32).astype(ml_dtypes.bfloat16)


def _chunk_x(xt_bf16):
    """[D, S] -> [p, si, c, s'] so per-si DMA is 8KB-contiguous/partition."""
    return np.ascontiguousarray(
        xt_bf16.reshape(DCH, 128, 4, 512).transpose(1, 2, 0, 3))


def _chunk_w(w_bf16):
    """[D, Ecols] -> [p, c, e]."""
    return np.ascontiguousarray(
        w_bf16.reshape(DCH, 128, -1).transpose(1, 0, 2))


def _shard_inputs(q, k, v, Wq, bq, Wk, bk, Wv, bv):
    """Build the 8 per-core input maps (host-side marshaling)."""
    xT = {}
    for b in range(B):
        xT[("q", b)] = _chunk_x(_to_bf16(np.asarray(q)[b].T))
        xT[("k", b)] = _chunk_x(_to_bf16(np.asarray(k)[b].T))
        xT[("v", b)] = _chunk_x(_to_bf16(np.asarray(v)[b].T))
    Wq, Wk, Wv = (np.asarray(a, np.float32) for a in (Wq, Wk, Wv))
    bq, bk, bv = (np.asarray(a, np.float32) for a in (bq, bk, bv))
    in_maps = []
    for c in range(NCORES):
        b, g = divmod(c, HPC)
        sl = slice(E * g, E * (g + 1))
        wv_p = np.zeros((D, EV), np.float32)
        bv_p = np.zeros((128, EV), np.float32)
        for h in range(HPC):
            wv_p[:, 65 * h:65 * h + HD] = \
                Wv[:, E * g + HD * h:E * g + HD * (h + 1)]
            bv_p[:, 65 * h:65 * h + HD] = \
                bv[E * g + HD * h:E * g + HD * (h + 1)]
            bv_p[:, 65 * h + HD] = 1.0
        in_maps.append({
            "xqT": xT[("q", b)], "xkT": xT[("k", b)], "xvT": xT[("v", b)],
            "wq": _chunk_w(_to_bf16(Wq[:, sl])),
            "wk": _chunk_w(_to_bf16(Wk[:, sl])),
            "wv": _chunk_w(_to_bf16(wv_p)),
            "bq": np.ascontiguousarray(bq[sl].reshape(2, 128).T),
            "bk": np.ascontiguousarray(bk[sl].reshape(2, 128).T),
            "bv": bv_p,
        })
    return in_maps


def kernel(q, k, v, Wq, bq, Wk, bk, Wv, bv):
    nc = _get_nc()
    in_maps = _shard_inputs(q, k, v, Wq, bq, Wk, bk, Wv, bv)
    res = run_bass_kernel_spmd(nc, in_maps, core_ids=list(range(NCORES)))
    outp = np.empty((B, S, D), np.float32)
    for c in range(NCORES):
        b, g = divmod(c, HPC)
        arr = res.results[c]["out"]  # [hh, p, t, d]
        outp[b, :, E * g:E * (g + 1)] = \
            arr.transpose(2, 1, 0, 3).reshape(S, E)
    return outp
